# revision 3
# baseline (speedup 1.0000x reference)
"""Trainium2 Bass kernel for the differentiable-Kalman-filter loss (v2).

Math: the reference runs a T=100000-step linear recurrence
  x_{i+1} = M x_i + K obs[i-1],  eps_i = obs[i] - C x_{i+1},  M = A - K C
and accumulates yvar = sum outer(eps_i) + decaying P-terms, loss = slogdet(yvar/T).
rho(M) ~ 0.95, so eps is a truncated causal convolution of obs.  Each core
computes eps for a 10240-row slab via a two-level blocked conv:
  - within-block (B=16 rows) taps as one column-trimmed 512x512 block-upper-
    triangular matmul whose matrix also carries the obs identity on the
    subdiagonal (eps = obs - conv in one pass),
  - block-boundary states from J=8 block-level taps (g-chain).
The Gram E^T E accumulates on-chip into a single [128,128] PSUM tile.
First W=18080 rows + the tiny P-series are computed exactly on host in f64.
"""
import numpy as np

T, N, B, J, W, NCORES = 100000, 32, 16, 8, 18080, 8
R = (T - W) // NCORES        # rows per core = 10240
NB = R // B                  # 640 blocks per core
PSI = NB + J                 # 648 panel columns (incl halo)
NTS = 5                      # s-tiles per core
CHW = [512, 416, 288, 160]   # conv chunk col widths (triangular trim)
COL0 = [0, 96, 224, 352]     # conv chunk output col bases
OFFS = [0, 512, 928, 1216]   # chunk col offsets inside trilh tile
TRW = 1376                   # trilh total cols
CS = 648                     # chunk stride in pans
HAL = 4 * CS                 # halo offset in pans; pans is [128, HAL+128]
NJUNK = 24                   # PE warmup matmuls (clock-ramp trigger)

_PROG_CACHE = {}


def _build_device_consts(A64, C64, K64):
    import ml_dtypes
    bf16 = ml_dtypes.bfloat16
    M = A64 - K64 @ C64
    Mp = [np.eye(N)]
    for _ in range(B + 1):
        Mp.append(M @ Mp[-1])
    H = [C64 @ Mp[k] @ K64 for k in range(B)]
    TrilX = np.zeros((512, 512))
    for r in range(B):
        for t in range(r, B):
            TrilX[r*N:(r+1)*N, t*N:(t+1)*N] = -H[t - r].T
    for t in range(B - 1):                      # obs identity fold
        TrilX[(t+1)*N:(t+2)*N, t*N:(t+1)*N] += np.eye(N)
    trilh = np.zeros((128, TRW))
    for kc in range(4):
        trilh[:, OFFS[kc]:OFFS[kc]+CHW[kc]] = TrilX[128*kc:128*kc+128, COL0[kc]:512]
    Gmat = np.zeros((512, N))
    for r in range(B):
        Gmat[r*N:(r+1)*N, :] = (Mp[B-1-r] @ K64).T
    gmat = np.ascontiguousarray(
        Gmat.reshape(4, 128, N).transpose(1, 0, 2).reshape(128, 128))
    MB = Mp[B]
    D = [np.eye(N)]
    for _ in range(J - 1):
        D.append(MB @ D[-1])
    dstk = np.zeros((128, 32 * (J // 4)))
    for jg in range(J // 4):
        for rho in range(4):
            dstk[32*rho:32*rho+32, 32*jg:32*jg+32] = D[4*jg + rho].T
    consts = np.concatenate([gmat, dstk], axis=1).astype(bf16)  # [128, 192]
    CMn = np.zeros((N, 512))
    for t in range(B):
        CMn[:, t*N:(t+1)*N] = -(C64 @ Mp[t+1]).T
    cmn_id = np.concatenate([CMn, np.eye(N)], axis=1).astype(bf16)  # [32, 544]
    return trilh.astype(bf16), consts, cmn_id


def _host_exact(obs, A64, C64, K64, x0, Psqrt0):
    """f64 exact: P-series + outer(obs0) + eps outers for i < W."""
    obs64 = obs.astype(np.float64)
    M = A64 - K64 @ C64
    Y = np.outer(obs64[0], obs64[0])
    P = Psqrt0.astype(np.float64)
    for _ in range(4000):
        CP = C64 @ P
        Y += CP @ CP.T
        P = M @ P
        if np.abs(P).max() < 1e-16:
            break
    x = x0.astype(np.float64)
    for i in range(W):
        o_prev = obs64[i - 1] if i > 0 else obs64[T - 1]
        x = M @ x + K64 @ o_prev
        eps = obs64[i] - C64 @ x
        Y += np.outer(eps, eps)
    return Y


def _patch_tile_drain():
    """This walrus build allows only one sem wait per Drain; split the
    TileContext tail drain's waits across multiple drain instructions."""
    import concourse.tile as tile
    from concourse.vector_clock import ScopedClock
    if getattr(tile.TileContext, "_kf_drain_patched", False):
        return
    def _drain_and_barrier(self, tick_clock, wait_clock):
        nc = self.nc
        drain_inst = nc.sync.drain()
        wait_clock.add_sem_waits(drain_inst.ins, ScopedClock({None: tick_clock.global_clock}))
        si = drain_inst.ins.sync_info
        waits = list(si.on_wait or [])
        if len(waits) > 1:
            si.on_wait = waits[:1]
            for i in range(1, len(waits)):
                extra = nc.sync.drain()
                esi = extra.ins.sync_info
                if esi is None:
                    extra.ins.sync_info = type(si)(on_wait=waits[i:i+1], on_update=[])
                else:
                    esi.on_wait = waits[i:i+1]
        nc.all_engine_barrier(sem_only=True)
        assert self.sems is not None
        popped = nc._tile_sem_poison_stack.pop()
        assert popped is self._sem_poison
        nc.clear_and_free_semaphores(list(self.sems.allocated().values()))
    tile.TileContext._drain_and_barrier = _drain_and_barrier
    tile.TileContext._kf_drain_patched = True


def _split_multi_waits(nc):
    """This walrus build encodes at most one sem wait per instruction; hoist
    extra waits onto NoOps inserted just before in the same engine stream."""
    import concourse.mybir as mybir
    for func in nc.m.functions:
        for blk in func.blocks:
            insts = blk.instructions
            out, changed = [], False
            for inst in insts:
                si = inst.sync_info
                waits = list(si.on_wait) if si and si.on_wait else []
                if len(waits) > 1:
                    changed = True
                    for k, w in enumerate(waits[:-1]):
                        out.append(mybir.InstNoOp(
                            name=f"{inst.name}-hw{k}", engine=inst.engine,
                            bass_nofuse=True,
                            sync_info=mybir.SyncInfo(on_wait=[w], on_update=[])))
                    si.on_wait = [waits[-1]]
                out.append(inst)
            if changed:
                blk.instructions = out


def build_program(debug=False):
    import concourse.bass as bass
    import concourse.mybir as mybir
    import concourse.tile as tile
    _patch_tile_drain()
    f32 = mybir.dt.float32
    bf16 = mybir.dt.bfloat16

    nc = bass.Bass()
    pans_in = nc.declare_dram_parameter("pans", [128, HAL + 128], bf16, isOutput=False)
    trilh_in = nc.declare_dram_parameter("trilh", [128, TRW], bf16, isOutput=False)
    consts_in = nc.declare_dram_parameter("consts", [128, 192], bf16, isOutput=False)
    cmn_id_in = nc.declare_dram_parameter("cmn_id", [32, 544], bf16, isOutput=False)
    yout = nc.declare_dram_parameter("yout", [128, 128], f32, isOutput=True)
    if debug:
        dbg_gt = nc.declare_dram_parameter("dbg_gt", [32, PSI], f32, isOutput=True)
        dbg_xbt = nc.declare_dram_parameter("dbg_xbt", [32, NB], f32, isOutput=True)
        dbg_e0 = nc.declare_dram_parameter("dbg_e0", [128, 512], f32, isOutput=True)

    with tile.TileContext(nc) as tc:
        with (
            tc.tile_pool(name="consts", bufs=1) as cpool,
            tc.tile_pool(name="work", bufs=1) as wpool,
            tc.tile_pool(name="etile", bufs=3) as epool,
            tc.tile_pool(name="trps", bufs=3, space="PSUM") as trpool,
            tc.tile_pool(name="epsum", bufs=3, space="PSUM") as eppool,
            tc.tile_pool(name="gramps", bufs=1, space="PSUM") as gpool,
        ):
            pans = cpool.tile([128, HAL + 128], bf16)
            trilh = cpool.tile([128, TRW], bf16)
            consts = cpool.tile([128, 192], bf16)
            cmn_id = cpool.tile([32, 544], bf16)
            wsrc = cpool.tile([128, 128], bf16)

            # warmup source; junk matmuls lift the HAM clock gate while DMAs run
            nc.gpsimd.memset(wsrc[:], 0.25)

            # ---- input DMAs: ring A (sync) pans, ring B (scalar) consts+trilh
            nc.sync.dma_start(pans[:, HAL:HAL+128], pans_in[:, HAL:HAL+128])
            nc.scalar.dma_start(consts[:], consts_in[:])
            for kc in range(4):
                nc.sync.dma_start(pans[:, kc*CS:(kc+1)*CS], pans_in[:, kc*CS:(kc+1)*CS])
            nc.scalar.dma_start(trilh[:], trilh_in[:])
            nc.scalar.dma_start(cmn_id[:], cmn_id_in[:])

            # ---- PE warmup: one accumulation group of junk matmuls
            warm = eppool.tile([128, 512], f32, tag="epsum")
            for i in range(NJUNK):
                nc.tensor.matmul(warm[:, 0:128], lhsT=wsrc[:], rhs=wsrc[:],
                                 start=(i == 0), stop=(i == NJUNK - 1),
                                 skip_group_check=True)

            # ---- g-chain: gth [32,J], gtm [32,NB] (split 512+128 for PSUM banks)
            gth_ps = trpool.tile([32, 32], f32, tag="trps")
            for kc in range(4):
                nc.tensor.matmul(gth_ps[:, 0:J],
                                 lhsT=consts[:, 32*kc:32*kc+32],
                                 rhs=pans[:, HAL + 32*kc : HAL + 32*kc + J],
                                 start=(kc == 0), stop=(kc == 3))
            gtm_a = trpool.tile([32, 512], f32, tag="trps")
            gtm_b = trpool.tile([32, 128], f32, tag="trps")
            for kc in range(4):
                nc.tensor.matmul(gtm_a[:, 0:512],
                                 lhsT=consts[:, 32*kc:32*kc+32],
                                 rhs=pans[:, kc*CS : kc*CS + 512],
                                 start=(kc == 0), stop=(kc == 3))
            for kc in range(4):
                nc.tensor.matmul(gtm_b[:, 0:128],
                                 lhsT=consts[:, 32*kc:32*kc+32],
                                 rhs=pans[:, kc*CS + 512 : kc*CS + NB],
                                 start=(kc == 0), stop=(kc == 3))

            # gts [32, PSI] then gss [128, PSI] via DVE partition copies
            gts = wpool.tile([32, PSI], bf16)
            nc.vector.tensor_copy(gts[:, 0:J], gth_ps[:, 0:J])
            nc.vector.tensor_copy(gts[:, J:J+512], gtm_a[:])
            nc.vector.tensor_copy(gts[:, J+512:PSI], gtm_b[:])
            gss = wpool.tile([128, PSI], bf16)
            for rho in range(4):
                nc.vector.tensor_copy(gss[32*rho:32*rho+32, rho:PSI],
                                      gts[:, 0:PSI-rho])

            # xbt [32, NB]
            xbt_a = trpool.tile([32, 512], f32, tag="trps")
            xbt_b = trpool.tile([32, 128], f32, tag="trps")
            for jg, j0 in enumerate(range(0, J, 4)):
                nc.tensor.matmul(xbt_a[:, 0:512],
                                 lhsT=consts[:, 128 + 32*jg : 128 + 32*jg + 32],
                                 rhs=gss[:, (J-1-j0) : (J-1-j0) + 512],
                                 start=(jg == 0), stop=(jg == J//4 - 1))
            for jg, j0 in enumerate(range(0, J, 4)):
                nc.tensor.matmul(xbt_b[:, 0:128],
                                 lhsT=consts[:, 128 + 32*jg : 128 + 32*jg + 32],
                                 rhs=gss[:, (J-1-j0) + 512 : (J-1-j0) + NB],
                                 start=(jg == 0), stop=(jg == J//4 - 1))
            xbt = wpool.tile([32, NB], bf16)
            nc.vector.tensor_copy(xbt[:, 0:512], xbt_a[:])
            nc.vector.tensor_copy(xbt[:, 512:NB], xbt_b[:])

            # ---- conv + gram; emission interleaved so PE never waits on xbt
            gram_ps = gpool.tile([128, 128], f32)
            eps_tiles = []

            def conv_chunks(st):
                ep = eppool.tile([128, 512], f32, tag="epsum")
                eps_tiles.append(ep)
                for kc in range(4):
                    nc.tensor.matmul(ep[:, COL0[kc]:512],
                                     lhsT=pans[:, kc*CS + 128*st : kc*CS + 128*st + 128],
                                     rhs=trilh[:, OFFS[kc]:OFFS[kc]+CHW[kc]],
                                     start=(kc == 0), stop=False)
                nc.tensor.matmul(ep[:, 480:512],
                                 lhsT=pans[0:32, 128*st + 1 : 128*st + 129],
                                 rhs=cmn_id[:, 512:544],
                                 start=False, stop=False)

            def finalize(st, first_gram, last_gram):
                ep = eps_tiles[st]
                nc.tensor.matmul(ep[:, 0:512],
                                 lhsT=xbt[:, 128*st:128*st+128],
                                 rhs=cmn_id[:, 0:512],
                                 start=False, stop=True)
                esb = epool.tile([128, 512], bf16, tag="etile")
                nc.vector.tensor_copy(esb[:], ep[:])
                if debug and st == 0:
                    nc.sync.dma_start(dbg_e0[:], esb[:])
                for g in range(4):
                    nc.tensor.matmul(gram_ps[:],
                                     lhsT=esb[:, 128*g:128*g+128],
                                     rhs=esb[:, 128*g:128*g+128],
                                     start=(first_gram and g == 0),
                                     stop=(last_gram and g == 3),
                                     skip_group_check=True)

            conv_chunks(0)
            conv_chunks(1)
            finalize(0, True, False)
            conv_chunks(2)
            finalize(1, False, False)
            conv_chunks(3)
            finalize(2, False, False)
            conv_chunks(4)
            finalize(3, False, False)
            finalize(4, False, True)

            ysb = wpool.tile([128, 128], f32)
            nc.vector.tensor_copy(ysb[:], gram_ps[:])
            nc.sync.dma_start(yout[:], ysb[:])
            if debug:
                nc.sync.dma_start(dbg_gt[:], gts[:])
                nc.sync.dma_start(dbg_xbt[:], xbt[:])

    _split_multi_waits(nc)
    return nc


def _core_inputs(obs, c, consts3):
    """Host-side layout prep for one core: transposed bf16 panels."""
    import ml_dtypes
    bf16 = ml_dtypes.bfloat16
    trilh, consts, cmn_id = consts3
    start = W + c * R
    hb = J * B + 1
    flat = obs[start - hb : start + R]
    pm = np.zeros((NB + 8, 512), np.float32)
    pm[:NB] = flat[hb - 1 : hb - 1 + R].reshape(NB, 512)
    pm[NB, 0:32] = flat[hb - 1 + R]            # panel row 0 of block NB
    pans = np.zeros((128, HAL + 128), np.float32)
    for kc in range(4):
        pans[:, kc*CS : kc*CS + NB + 8] = pm[:, 128*kc:128*kc+128].T
    ph = flat[0 : J * B].reshape(J, 512)       # halo panels (blocks -J..-1)
    for kc in range(4):
        pans[:, HAL + 32*kc : HAL + 32*kc + J] = ph[:, 128*kc:128*kc+128].T
    pans = np.ascontiguousarray(pans).astype(bf16)
    return {"pans": pans, "trilh": trilh, "consts": consts, "cmn_id": cmn_id}


def kernel(observations, A, C, K, x0, Psqrt0, _trace=False, _trace_kwargs=None):
    obs = np.ascontiguousarray(observations, np.float32)
    A64 = np.asarray(A, np.float64)
    C64 = np.asarray(C, np.float64)
    K64 = np.asarray(K, np.float64)

    consts3 = _build_device_consts(A64, C64, K64)
    Y = _host_exact(obs, A64, C64, K64, np.asarray(x0), np.asarray(Psqrt0))

    if "prog" not in _PROG_CACHE:
        _PROG_CACHE["prog"] = build_program()
    nc = _PROG_CACHE["prog"]

    in_maps = [_core_inputs(obs, c, consts3) for c in range(NCORES)]

    from concourse.bass_utils import run_bass_kernel_spmd
    kw = dict(_trace_kwargs or {})
    res = run_bass_kernel_spmd(nc, in_maps, list(range(NCORES)), trace=_trace, **kw)

    for c in range(NCORES):
        G = np.asarray(res.results[c]["yout"], np.float64)
        for tau in range(4):
            Y += G[32*tau:32*tau+32, 32*tau:32*tau+32]
    loss = np.linalg.slogdet(Y / T)[1]
    out = np.float32(loss)
    if _trace:
        return out, res
    return out
